# revision 49
# baseline (speedup 1.0000x reference)
"""Causal multi-head attention block (B=2, L=2048, D=1024, H=16) on 8 trn2 cores.

Sharding: core c -> batch b = c // 4, head group g = c % 4 (heads 4g..4g+4).
Per core (bf16 matmul operands, fp32 PSUM accumulate):
  1. QT/KT = (W_qk x^T)          (d_head on partitions; 512 x 2048 each core)
     K bias dropped (softmax-invariant); Q bias added on DVE.
  2. V     = (x W_v^T + b_v)     (j on partitions; two ones-columns per head
                                  accumulate the softmax denominators)
  3. per (head, 512-query-block): ST = K Q^T (j-part, i-free), causal-narrowed
     diagonal chunks; psum groups opened by a 128-col mask-generating matmul
     (sum_k u[k,jr] w[k,ic] = -1e30*max(0, jr-ic)); P^T = exp(0.125 ST)
     (no-max softmax; scores ~N(0,1)), O^T|l = [V|1]^T P^T, normalized by 1/l
     (fast-approx reciprocal of a matmul-broadcast denominator row)
  4. y^T_partial = W_out,local O^T (PSUM -> SBUF -> DMA, bf16)
Host: y[b] = sum of the 4 partials^T + b_out.
Schedule: upfront K/V/Q of x-block 0, then attention blocks t=0..3 with
deferred projection/QKV units as PE fillers sized to each block's exp load
(keeps the PE dense so the HAM clock stays at 8/8).
"""

import numpy as np
import ml_dtypes

import concourse.bass as bass
import concourse.bacc as bacc
import concourse.mybir as mybir
from concourse.tile import TileContext
from concourse.bass_utils import run_bass_kernel_spmd

B, L, D, H = 2, 2048, 1024, 16
HD = 64                      # head dim
HPC = 4                      # heads per core
DL = HPC * HD                # 256 local head dims
N_CORES = 8
NEG = -1.0e30
SCALE = 1.0 / 8.0            # 1/sqrt(64)
FP32 = mybir.dt.float32
FP32R = mybir.dt.float32r
BF16 = mybir.dt.bfloat16
AF = mybir.ActivationFunctionType
BF = ml_dtypes.bfloat16

NKC = D // 128               # 8 contraction chunks over D
NMB = L // 512               # 4 column blocks of 512 over L
NJC = L // 128               # 16 j-chunks of 128


def r32(ap):
    return ap.bitcast(FP32R)


def build_program():
    nc = bacc.Bacc("TRN2", target_bir_lowering=False, debug=False)

    xt = nc.dram_tensor("xt", [D, L], BF16, kind="ExternalInput")
    wqk = nc.dram_tensor("wqk", [D, 2 * DL], BF16, kind="ExternalInput")
    wv = nc.dram_tensor("wv", [D, DL], BF16, kind="ExternalInput")
    wout = nc.dram_tensor("wout", [DL, D], BF16, kind="ExternalInput")
    bq = nc.dram_tensor("bq", [DL, 1], FP32, kind="ExternalInput")
    bv = nc.dram_tensor("bv", [1, DL], FP32, kind="ExternalInput")
    trid = nc.dram_tensor("trid", [128, 128], BF16, kind="ExternalInput")
    seld = nc.dram_tensor("seld", [34, 128], FP32, kind="ExternalInput")
    yt = nc.dram_tensor("yt", [D, L], BF16, kind="ExternalOutput")

    with TileContext(nc) as tc:
        with (
            tc.tile_pool(name="const", bufs=1) as const,
            tc.tile_pool(name="xtp", bufs=16) as xtp,
            tc.tile_pool(name="ptp", bufs=6) as ptp,
            tc.tile_pool(name="rp", bufs=3) as rp,
            tc.tile_pool(name="yp", bufs=4) as yp,
            tc.tile_pool(name="ps_st", bufs=2, space="PSUM") as ps_st,
            tc.tile_pool(name="ps_ot", bufs=2, space="PSUM") as ps_ot,
            tc.tile_pool(name="ps_sm", bufs=2, space="PSUM") as ps_sm,
        ):
            # ---- persistent constants / weights ----
            # (DMA issue order matters: the first K-unit needs wqk + x block 0.
            # Those are queued first, spread across 4 DGE queues — descriptor
            # issue is ~0.7us per dma_start per engine, so one queue would
            # pace the first qk unit at DMA-issue rate.)
            dma_engs = (nc.sync, nc.scalar, nc.gpsimd)
            wqk_t = []
            xts0 = []
            for kc in range(NKC):
                t = const.tile([128, 2 * DL], BF16, tag=f"wqk{kc}")
                dma_engs[(2 * kc) % 3].dma_start(
                    out=t[:], in_=wqk[kc * 128:(kc + 1) * 128, :])
                wqk_t.append(t)
                tx = xtp.tile([128, 512], BF16, name="t")
                dma_engs[(2 * kc + 1) % 3].dma_start(
                    out=tx[:], in_=xt[kc * 128:(kc + 1) * 128, 0:512])
                xts0.append(tx)
            bq_t = []
            for nt in range(2):
                t = const.tile([128, 1], FP32, tag=f"bq{nt}")
                nc.sync.dma_start(out=t[:], in_=bq[nt * 128:(nt + 1) * 128, :])
                bq_t.append(t)

            def load_consts2():
                wv_t = []
                for kc in range(NKC):
                    t = const.tile([128, DL], BF16, tag=f"wv{kc}")
                    dma_engs[kc % 3].dma_start(
                        out=t[:], in_=wv[kc * 128:(kc + 1) * 128, :])
                    wv_t.append(t)
                bvrep = const.tile([128, DL], FP32, tag="bvrep")
                nc.sync.dma_start(out=bvrep[:],
                                  in_=bv[0:1, :].to_broadcast((128, DL)))
                sel_t = const.tile([34, 128], FP32R, tag="sel")
                nc.sync.dma_start(out=sel_t[:], in_=r32(seld[:, :]))
                tri_t = const.tile([128, 128], BF16, tag="tri")
                nc.gpsimd.dma_start(out=tri_t[:], in_=trid[:, :])
                return wv_t, bvrep, sel_t, tri_t

            def load_consts3():
                wout_t = []
                for n2 in range(2):
                    t = const.tile([128, D], BF16, tag=f"wout{n2}")
                    nc.sync.dma_start(out=t[:],
                                      in_=wout[n2 * 128:(n2 + 1) * 128, :])
                    wout_t.append(t)
                return wout_t

            # persistent activations
            # qk_t[0..1]: QT tiles (128 rows each: heads {2i,2i+1}); qk_t[2..3]: KT
            qk_t = [const.tile([128, L], BF16, tag=f"qk{nt}", name=f"qk{nt}")
                    for nt in range(4)]
            # V tiles per j-chunk: [128, 4*66]; head h cols h*66..h*66+64 = V,
            # cols h*66+64..66 = 1.0 (denominator accumulator columns, set
            # once here — DVE is idle during the preamble)
            v_t = [const.tile([128, 4 * 66], BF16, tag=f"v{j}", name=f"v{j}")
                   for j in range(NJC)]
            for j in range(NJC):
                v4j = v_t[j][:].rearrange("p (h m) -> p h m", m=66)
                nc.vector.memset(v4j[:, :, 64:66], 1.0)
            ot_t = [const.tile([128, L], BF16, tag=f"ot{n2}", name=f"ot{n2}")
                    for n2 in range(2)]
            # denominator staging rows for the selector broadcast: rows 0 and
            # 32 are written per use; the rest stay zero (memset once here)
            ls_t = [const.tile([34, 512], FP32R, tag=f"ls{i}", name=f"ls{i}")
                    for i in range(2)]
            for t in ls_t:
                nc.vector.memset(t[:].bitcast(FP32), 0.0)

            def load_x(m):
                xts = []
                for kc in range(NKC):
                    t = xtp.tile([128, 512], BF16)
                    nc.sync.dma_start(
                        out=t[:],
                        in_=xt[kc * 128:(kc + 1) * 128, m * 512:(m + 1) * 512])
                    xts.append(t)
                return xts

            def qk_unit(xts, m, nt):
                # nt 0,1: Q rows (bias added on DVE); nt 2,3: K rows (no bias)
                ps = ps_sm.tile([128, 512], FP32, tag="ps_sm")
                for kc in range(NKC):
                    nc.tensor.matmul(
                        ps[:],
                        wqk_t[kc][:, nt * 128:(nt + 1) * 128],
                        xts[kc][:],
                        start=(kc == 0), stop=(kc == NKC - 1))
                isl = slice(m * 512, (m + 1) * 512)
                with nc.allow_low_precision(reason="bf16 activations"):
                    if nt < 2:
                        nc.vector.tensor_scalar_add(
                            qk_t[nt][:, isl], ps[:], bq_t[nt][:])
                    else:
                        nc.vector.tensor_copy(qk_t[nt][:, isl], ps[:])

            def v_unit(xts, m, ic):
                j = 4 * m + ic
                ps = ps_sm.tile([128, 512], FP32, tag="ps_sm")
                for kc in range(NKC):
                    nc.tensor.matmul(
                        ps[:, 0:DL],
                        xts[kc][:, ic * 128:(ic + 1) * 128],
                        wv_t[kc][:],
                        start=(kc == 0), stop=(kc == NKC - 1))
                v4 = v_t[j][:].rearrange("p (h m) -> p h m", m=66)
                with nc.allow_low_precision(reason="bf16 activations"):
                    nc.vector.tensor_add(
                        v4[:, :, 0:64],
                        ps[:, 0:DL].rearrange("p (h d) -> p h d", d=64),
                        bvrep[:].rearrange("p (h d) -> p h d", d=64))

            def attn_pair(hs, t_, filler):
                """Two heads' ST -> exp -> PV chains. Each j-chunk gets one
                pair-shared PSUM tile [128, 1024]: head A's scores in cols
                0:512 (bank 0), head B's in 512:1024 (bank 1). The two heads'
                score matmuls use partitions 0:64 / 64:128 so they run
                concurrently in separate PE row-groups, and each chunk's exp
                frees its tile independently (finer pipelining than per-head
                2-chunk tiles)."""
                n_j = 4 * (t_ + 1)
                otps = [ps_ot.tile([128, 512], FP32, tag="ps_ot", name=f"otp{h}")
                        for h in hs]
                for jp in range(0, n_j, 2):
                    stps = [ps_st.tile([128, 1024], FP32, tag="ps_st",
                                       name=f"stp{jj}") for jj in range(2)]
                    w0s = []
                    for jj in range(2):
                        q = jp + jj - 4 * t_
                        w0s.append(128 * q if q > 0 else 0)
                    # scores, head-paired per chunk for row-group concurrency.
                    # Diagonal 128x128 blocks are computed unmasked; the upper
                    # triangle of P is zeroed on DVE after the exp.
                    for jj in range(2):
                        J = jp + jj
                        w0 = w0s[jj]
                        for i, h in enumerate(hs):
                            qt = qk_t[h // 2]
                            kt = qk_t[2 + h // 2]
                            po = (h % 2) * 64
                            nc.tensor.matmul(
                                stps[jj][:, i * 512 + w0:(i + 1) * 512],
                                kt[po:po + 64, J * 128:(J + 1) * 128],
                                qt[po:po + 64, t_ * 512 + w0:(t_ + 1) * 512],
                                start=True, stop=True)
                    # exp per chunk (covers both heads' banks in one call on
                    # off-diagonal chunks)
                    ptiles = []
                    for jj in range(2):
                        J = jp + jj
                        ptile = ptp.tile([128, 1024], BF16, name=f"pt{jj}")
                        w0 = w0s[jj]
                        if w0 > 0:
                            for i in range(2):
                                nc.scalar.activation(
                                    ptile[:, i * 512 + w0:(i + 1) * 512],
                                    stps[jj][:, i * 512 + w0:(i + 1) * 512],
                                    AF.Exp, scale=SCALE)
                        else:
                            nc.scalar.activation(ptile[:], stps[jj][:],
                                                 AF.Exp, scale=SCALE)
                        if J - 4 * t_ >= 0:
                            # causal fixup: P[jr, w0+ic] *= [jr <= ic] on the
                            # diagonal 128x128 block of each head
                            with nc.allow_low_precision(reason="bf16 P"):
                                for i in range(2):
                                    c0 = i * 512 + w0
                                    nc.vector.tensor_mul(
                                        ptile[:, c0:c0 + 128],
                                        ptile[:, c0:c0 + 128], tri_t[:])
                        ptiles.append(ptile)
                    filler()
                    for jj in range(2):
                        J = jp + jj
                        w0 = w0s[jj]
                        for i, h in enumerate(hs):
                            nc.tensor.matmul(
                                otps[i][0:66, w0:512],
                                v_t[J][:, h * 66:(h + 1) * 66],
                                ptiles[jj][:, i * 512 + w0:(i + 1) * 512],
                                start=(J == 0), stop=(J == n_j - 1))
                return otps

            def norm_stage(otps, ls):
                """Copy both heads' O^T|l out of PSUM (releases PV slots) and
                stack the denominator rows for the selector matmul."""
                osb = rp.tile([128, 512], FP32, name="osb")
                for i, otp in enumerate(otps):
                    nc.vector.tensor_copy(osb[64 * i:64 * i + 64, :],
                                          otp[0:64, :])
                    with nc.allow_low_precision(reason="fp32r matmul operand"):
                        nc.vector.tensor_copy(ls[32 * i:32 * i + 1, :],
                                              otp[64:65, :])
                return osb

            def norm_finish(osb, ls, hs, t_):
                """Broadcast denominators, approx-reciprocal, scale into ot_t.
                Deferred into the next pair's filler slot so the rb matmul's
                wait overlaps the next pair's score matmuls."""
                isl = slice(t_ * 512, (t_ + 1) * 512)
                rb = ps_sm.tile([128, 512], FP32, tag="ps_sm")
                nc.tensor.matmul(rb[:], sel_t[:], ls[:], start=True, stop=True)
                rbb = rp.tile([128, 512], FP32, name="rbb")
                nc.vector.reciprocal_approx_fast(out=rbb[:], in_=rb[:])
                with nc.allow_low_precision(reason="bf16 activations"):
                    for i, h in enumerate(hs):
                        nc.vector.tensor_mul(
                            ot_t[h // 2][(h % 2) * 64:(h % 2) * 64 + 64, isl],
                            osb[64 * i:64 * i + 64, :],
                            rbb[64 * i:64 * i + 64, :])

            def proj_unit(t_, dt_, tail=False):
                isl = slice(t_ * 512, (t_ + 1) * 512)
                ps = ps_sm.tile([128, 512], FP32, tag="ps_sm")
                for n2 in range(2):
                    nc.tensor.matmul(
                        ps[:],
                        wout_t[n2][:, dt_ * 128:(dt_ + 1) * 128],
                        ot_t[n2][:, isl],
                        start=(n2 == 0), stop=(n2 == 1))
                ys = yp.tile([128, 512], BF16, name="ys")
                with nc.allow_low_precision(reason="bf16 output"):
                    if tail and dt_ % 2 == 0:
                        # kernel tail: ACT is idle, split copies across engines
                        nc.scalar.activation(ys[:], ps[:], AF.Copy)
                    else:
                        nc.vector.tensor_copy(ys[:], ps[:])
                # spread output DMAs over the DGE queues: descriptor issue is
                # ~0.7us per dma_start per engine and gates the kernel tail
                dma_engs[dt_ % 3].dma_start(
                    out=yt[dt_ * 128:(dt_ + 1) * 128, isl], in_=ys[:])

            # ---- program ----
            wv_t, bvrep, sel_t, tri_t = load_consts2()
            # upfront block 0: K first (scores need it), then V, then Q
            for nt in (2, 3):
                qk_unit(xts0, 0, nt)
            for u in range(4):
                v_unit(xts0, 0, u)
            for nt in (0, 1):
                qk_unit(xts0, 0, nt)
            wout_t = load_consts3()

            # Filler plans: block t gets the K/V/Q units of x-block t+1
            # (ready exactly when needed; K/V(m) must complete before block m
            # starts, Q(m) before block m). Block 3 gets the deferred
            # projections of blocks 0..2; projection of block 3 trails.
            def kvq_units(m):
                xts = load_x(m)
                us = []
                for nt in (2, 3):
                    us.append(lambda nt=nt, xts=xts, m=m: qk_unit(xts, m, nt))
                for u in range(4):
                    us.append(lambda u=u, xts=xts, m=m: v_unit(xts, m, u))
                for nt in (0, 1):
                    us.append(lambda nt=nt, xts=xts, m=m: qk_unit(xts, m, nt))
                return us

            for t_ in range(4):
                if t_ < 3:
                    units = kvq_units(t_ + 1)
                else:
                    units = [lambda pm=pm, dt_=dt_: proj_unit(pm, dt_)
                             for pm in (0, 1, 2) for dt_ in range(8)]

                n_slots = 4 * (t_ + 1) + 4
                state = {"i": 0, "acc": 0.0, "per": len(units) / n_slots}

                def filler(state=state, units=units):
                    state["acc"] += state["per"]
                    while state["i"] < min(state["acc"], len(units)):
                        units[state["i"]]()
                        state["i"] += 1

                def filler2(state=state):
                    if state.get("pending") is not None:
                        fin = state["pending"]
                        state["pending"] = None
                        fin()
                    filler()

                for hp in range(2):
                    hs = (2 * hp, 2 * hp + 1)
                    otps = attn_pair(hs, t_, filler2)
                    ls = ls_t[hp]
                    osb = norm_stage(otps, ls)
                    state["pending"] = (
                        lambda osb=osb, ls=ls, hs=hs: norm_finish(osb, ls, hs, t_))
                    filler()
                if state.get("pending") is not None:
                    state["pending"]()
                    state["pending"] = None
                while state["i"] < len(units):
                    units[state["i"]]()
                    state["i"] += 1

            for dt_ in range(8):
                proj_unit(3, dt_, tail=True)

    nc.compile()
    return nc


_NC_CACHE = None


def _get_nc():
    global _NC_CACHE
    if _NC_CACHE is None:
        _NC_CACHE = build_program()
    return _NC_CACHE


def make_in_maps(x, W_qkv, b_qkv, W_out):
    """Per-core input dicts (core c -> batch c//4, head group c%4)."""
    jr = np.arange(128)[:, None]
    ic = np.arange(128)[None, :]
    trid = (jr <= ic).astype(BF)   # [jr, ic]: 1 where key <= query (keep)

    in_maps = []
    for c in range(N_CORES):
        b, g = divmod(c, 4)
        rs = slice(DL * g, DL * g + DL)
        wq = W_qkv[0 * D:1 * D][rs]
        wk = W_qkv[1 * D:2 * D][rs]
        wv = W_qkv[2 * D:3 * D][rs]
        in_maps.append({
            "xt": np.ascontiguousarray(x[b].T).astype(BF),
            "wqk": np.ascontiguousarray(
                np.concatenate([wq, wk], 0).T).astype(BF),
            "wv": np.ascontiguousarray(wv.T).astype(BF),
            "wout": np.ascontiguousarray(W_out[:, rs].T).astype(BF),
            "bq": np.ascontiguousarray(
                b_qkv[0 * D:1 * D][rs][:, None], np.float32),
            "bv": np.ascontiguousarray(b_qkv[2 * D:3 * D][rs][None, :], np.float32),
            "trid": trid,
            "seld": np.concatenate([
                np.repeat(np.eye(2, dtype=np.float32), 64, axis=1)[0:1],
                np.zeros((31, 128), np.float32),
                np.repeat(np.eye(2, dtype=np.float32), 64, axis=1)[1:2],
                np.zeros((1, 128), np.float32)]),
        })
    return in_maps


def assemble_output(results, b_out):
    y = np.zeros((B, L, D), np.float32)
    for c in range(N_CORES):
        b = c // 4
        y[b] += results[c]["yt"].T.astype(np.float32)
    y += b_out[None, None, :].astype(np.float32)
    return y


def run(x, mask, W_qkv, b_qkv, W_out, b_out, trace=False, **spmd_kwargs):
    causal = np.array_equal(
        np.asarray(mask).reshape(L, L),
        np.triu(np.ones((L, L), bool), k=1))
    if not causal:
        # Fallback (never expected): reference semantics on host.
        print("WARNING: non-causal mask; computing on host")
        q, k, v = np.split(x @ W_qkv.T + b_qkv, 3, axis=-1)
        th = lambda t: t.reshape(B, L, H, HD).transpose(0, 2, 1, 3)
        q, k, v = th(q), th(k), th(v)
        a = np.einsum('bhqd,bhkd->bhqk', q, k) * SCALE
        a = np.where(np.asarray(mask), -np.inf, a)
        a = a - a.max(-1, keepdims=True)
        a = np.exp(a)
        a /= a.sum(-1, keepdims=True)
        o = np.einsum('bhqk,bhkd->bhqd', a, v)
        o = o.transpose(0, 2, 1, 3).reshape(B, L, D)
        return o @ W_out.T + b_out, None

    nc = _get_nc()
    in_maps = make_in_maps(np.asarray(x), np.asarray(W_qkv),
                           np.asarray(b_qkv), np.asarray(W_out))
    res = run_bass_kernel_spmd(nc, in_maps, list(range(N_CORES)),
                               trace=trace, **spmd_kwargs)
    y = assemble_output(res.results, np.asarray(b_out))
    return y, res


def kernel(x, mask, W_qkv, b_qkv, W_out, b_out):
    y, _ = run(x, mask, W_qkv, b_qkv, W_out, b_out)
    return y
